# revision 6
# baseline (speedup 1.0000x reference)
"""Trainium2 Bass kernel for a 3-layer ResGatedGraphConv GNN (ClinicalGatedGCN).

Strategy (8 NeuronCores, SPMD), v2:
  - Nodes partitioned into 8 contiguous ranges (rank-blocked, padded to 128).
    Edges assigned to the rank owning their dst node, sorted by
    (rank, src-epoch, dst-group, dst) on the host.
  - Each rank computes the full [q|v] node table into local HBM (split at row
    32768 so int16 gather indices reach every row in two epochs). One
    dma_gather per (epoch, dst-group) fetches the src rows of [q|v] — this is
    the ONLY GpSimd gather; k[dst] is expanded on the PE via a host-shipped
    0/1 dst-selector S_T (matmul S_T.T @ k_group), with attr*We folded into
    the same PSUM accumulation via a K=1 rank-1 matmul.
  - The segment-sum over dst is a PE matmul against the host-shipped selector
    S (edge-major). S/S_T depend only on the edge structure, so they are
    built once on the host and streamed from DRAM each layer.
  - Gather sizes are exact per (epoch, group): nt = ceil(max-over-ranks
    count / 128) tiles, carried in meta (shapes shared across the SPMD
    program).
  - The qv table is built hi-rows-first each layer and epoch-1 gathers are
    issued before the lo rows are written, so Q7 descriptor generation
    overlaps table construction.
  - h stays feature-major; per layer the updated h slice is AllGather'd.
    Mean-pool per graph is a matmul against a host-built indicator with
    1/cnt folded in; partial pools are AllGather'd and summed; the tiny
    classifier runs on every core.
"""

import numpy as np
import ml_dtypes

import concourse.bacc as bacc
import concourse.bass as bass
import concourse.mybir as mybir
import concourse.tile as tile
from concourse.bass_utils import run_bass_kernel_spmd
from concourse.masks import make_identity

F32 = mybir.dt.float32
BF16 = mybir.dt.bfloat16
I16 = mybir.dt.int16
AF = mybir.ActivationFunctionType
OP = mybir.AluOpType

# ---------------- problem constants (hardcoded per spec) ----------------
N, E, H, G, NCLIN, NCLS = 50000, 800000, 128, 64, 16, 2
NLAYER = 3
EPS = 1e-5
SLOPE = 0.01
R = 8                      # ranks / NeuronCores
SPLIT = 32768              # int16 gather index limit -> 2 epochs

USE_BF16 = True            # table/h/gate dtype

NPR = (N + R - 1) // R     # real nodes per rank
NGRP = (NPR + 127) // 128  # 128-node groups per rank
NPAD = NGRP * 128          # padded nodes per rank
NTOT = R * NPAD            # rank-blocked total rows


def _np_dtab(use_bf16):
    return ml_dtypes.bfloat16 if use_bf16 else np.float32


def wrap_idxs_block(idx):
    """Wrap one gather call's indices: idx j -> [j%16, j//16], tiled to 128 parts."""
    n = len(idx)
    assert n % 16 == 0
    w = np.asarray(idx, np.int16).reshape(n // 16, 16).T
    return np.tile(w, (8, 1))


# ---------------------------------------------------------------------------
# host-side preprocessing
# ---------------------------------------------------------------------------

def prep(inputs, use_bf16=None):
    if use_bf16 is None:
        use_bf16 = USE_BF16
    dtab = _np_dtab(use_bf16)
    x = np.asarray(inputs["x"], np.float32)
    edge_index = np.asarray(inputs["edge_index"])
    edge_attr = np.asarray(inputs["edge_attr"], np.float32)[:, 0]
    batch = np.asarray(inputs["batch"]).astype(np.int64)
    clinical = np.asarray(inputs["clinical"], np.float32)
    Wk, bk = np.asarray(inputs["Wk"], np.float32), np.asarray(inputs["bk"], np.float32)
    Wq, bq = np.asarray(inputs["Wq"], np.float32), np.asarray(inputs["bq"], np.float32)
    Wv, bv = np.asarray(inputs["Wv"], np.float32), np.asarray(inputs["bv"], np.float32)
    Ws, bs = np.asarray(inputs["Ws"], np.float32), np.asarray(inputs["bs"], np.float32)
    We, be = np.asarray(inputs["We"], np.float32), np.asarray(inputs["be"], np.float32)
    gamma = np.asarray(inputs["gamma"], np.float32)
    beta = np.asarray(inputs["beta"], np.float32)
    rmean = np.asarray(inputs["rmean"], np.float32)
    rvar = np.asarray(inputs["rvar"], np.float32)
    Wc, bc = np.asarray(inputs["Wc"], np.float32), np.asarray(inputs["bc"], np.float32)

    src = edge_index[0].astype(np.int64)
    dst = edge_index[1].astype(np.int64)

    # BN folded: A*x + B
    A = gamma / np.sqrt(rvar + EPS)
    B = beta - rmean * A
    bgate = bk + bq + be          # folded into k table bias
    rb_row = (src // NPR) * NPAD + (src % NPR)

    e_rank = dst // NPR
    epoch = (rb_row >= SPLIT).astype(np.int64)
    dst_local = dst - e_rank * NPR
    group = dst_local // 128
    dst_rel = dst_local % 128

    # per (ep, rank, group) counts -> exact per-(ep,group) tile counts
    counts = np.zeros((2, R, NGRP), np.int64)
    np.add.at(counts, (epoch, e_rank, group), 1)
    nt_l = [np.ceil(counts[ep].max(axis=0) / 128).astype(int) for ep in (0, 1)]
    off_l = [np.concatenate([[0], np.cumsum(nt)]).astype(int) for nt in nt_l]

    # graph counts for mean pooling
    cntg = np.bincount(batch, minlength=G).astype(np.float32)
    inv_cnt = 1.0 / np.maximum(cntg, 1.0)

    order = np.lexsort((dst, group, epoch, e_rank))
    src_s, dst_rel_s, attr_s = rb_row[order], dst_rel[order], edge_attr[order]
    ep_s, rank_s, grp_s = epoch[order], e_rank[order], group[order]
    key = ((rank_s * 2 + ep_s) * NGRP + grp_s)
    starts = np.searchsorted(key, np.arange(R * 2 * NGRP + 1))

    bias_k = np.zeros((NLAYER, 128, H), np.float32)
    for l in range(NLAYER):
        bias_k[l, :, :] = bgate[l][None, :]
    bias_v = np.zeros((NLAYER, 128, 2 * H), np.float32)
    for l in range(NLAYER):
        bias_v[l, :, H:2 * H] = bv[l][None, :]
    has_bias_k = bool(np.any(bias_k != 0))
    has_bias_qv = bool(np.any(bias_v != 0))
    has_bs = bool(np.any(bs != 0))
    has_bc = bool(np.any(bc != 0))
    We_row = We[:, 0, :].reshape(NLAYER, 1, H)

    x_rb = np.zeros((R * 128, NPAD), np.float32)
    for r in range(R):
        lo, hi = r * NPR, min((r + 1) * NPR, N)
        x_rb[r * 128:(r + 1) * 128, 0:hi - lo] = x[lo:hi].T

    in_maps = []
    for r in range(R):
        ep_arrs = {}
        for ep in (0, 1):
            nt = nt_l[ep]
            off = off_l[ep]
            tot_tiles = int(off[-1])
            slots = tot_tiles * 128
            gidx = np.zeros((128, tot_tiles * 8), np.int16)
            S = np.zeros((128, slots), dtab)
            ST = np.zeros((128, slots), dtab)
            attr_row = np.zeros((1, slots), dtab)
            for g in range(NGRP):
                ntg = int(nt[g])
                if ntg == 0:
                    continue
                k = (r * 2 + ep) * NGRP + g
                s0 = int(starts[k])
                n = int(counts[ep, r, g])
                o = int(off[g])
                idx = np.zeros((ntg * 128,), np.int64)
                idx[:n] = src_s[s0:s0 + n] - ep * SPLIT
                gidx[:, o * 8:(o + ntg) * 8] = wrap_idxs_block(idx)
                j = np.arange(n)
                t = j // 128
                p = j % 128
                drel = dst_rel_s[s0:s0 + n].astype(np.int64)
                col = (o + t) * 128
                S[p, col + drel] = 1
                ST[drel, col + p] = 1
                attr_row[0, o * 128 + j] = attr_s[s0:s0 + n].astype(dtab)
            ep_arrs[ep] = (gidx, S, ST, attr_row)
        IndT = np.zeros((NPAD, G), np.float32)
        lo, hi = r * NPR, min((r + 1) * NPR, N)
        IndT[np.arange(hi - lo), batch[lo:hi]] = inv_cnt[batch[lo:hi]]
        im = {
            "x_rb": x_rb.astype(dtab),
            "xT_loc": x_rb[r * 128:(r + 1) * 128].astype(dtab),
            "Wk": Wk.astype(dtab), "Wq": Wq.astype(dtab), "Wv": Wv.astype(dtab),
            "Ws": Ws.astype(dtab),
            "We_row": We_row.astype(dtab),
            "bias_k": bias_k,
            "bias_qv": bias_v,
            "bs_col": bs.reshape(NLAYER, H, 1),
            "A_col": A.reshape(NLAYER, H, 1),
            "B_col": B.reshape(NLAYER, H, 1),
            "gidx0": ep_arrs[0][0], "S0": ep_arrs[0][1], "ST0": ep_arrs[0][2],
            "attr0": ep_arrs[0][3],
            "gidx1": ep_arrs[1][0], "S1": ep_arrs[1][1], "ST1": ep_arrs[1][2],
            "attr1": ep_arrs[1][3],
            "IndT": IndT.astype(dtab),
            "clinT": clinical.T.copy(),
            "Wc_h": Wc[0:H], "Wc_c": Wc[H:H + NCLIN],
            "bc_rep": np.tile(bc, (G, 1)),
        }
        in_maps.append(im)
    meta = dict(NT0=tuple(int(v) for v in nt_l[0]),
                NT1=tuple(int(v) for v in nt_l[1]),
                has_bias_k=has_bias_k, has_bias_qv=has_bias_qv,
                has_bs=has_bs, has_bc=has_bc, use_bf16=use_bf16)
    return in_maps, meta


# ---------------------------------------------------------------------------
# device program
# ---------------------------------------------------------------------------

def build(meta):
    use_bf16 = meta["use_bf16"]
    parts = meta.get("parts", 4)
    DT = BF16 if use_bf16 else F32
    NT_L = [list(meta["NT0"]), list(meta["NT1"])]
    OFF_L = [np.concatenate([[0], np.cumsum(nt)]).astype(int) for nt in NT_L]
    TOT = [int(o[-1]) for o in OFF_L]
    TMAX = max(max(NT_L[0]), max(NT_L[1]))
    KQC = 4                           # kq psum chunk (tiles, 1 PSUM bank)

    nc = bacc.Bacc("TRN2", target_bir_lowering=False, debug=False, num_devices=R)

    def din(name, shape, dt):
        return nc.dram_tensor(name, shape, dt, kind="ExternalInput").ap()

    t_x_rb = din("x_rb", [R * 128, NPAD], DT)
    t_xT_loc = din("xT_loc", [128, NPAD], DT)
    t_Wk = din("Wk", [NLAYER, H, H], DT)
    t_Wq = din("Wq", [NLAYER, H, H], DT)
    t_Wv = din("Wv", [NLAYER, H, H], DT)
    t_Ws = din("Ws", [NLAYER, H, H], DT)
    t_We_row = din("We_row", [NLAYER, 1, H], DT)
    t_bias_k = din("bias_k", [NLAYER, 128, H], F32)
    t_bias_qv = din("bias_qv", [NLAYER, 128, 2 * H], F32)
    t_bs = din("bs_col", [NLAYER, H, 1], F32)
    t_A = din("A_col", [NLAYER, H, 1], F32)
    t_B = din("B_col", [NLAYER, H, 1], F32)
    t_gidx = [din("gidx0", [128, TOT[0] * 8], I16),
              din("gidx1", [128, TOT[1] * 8], I16)]
    t_S = [din("S0", [128, TOT[0] * 128], DT),
           din("S1", [128, TOT[1] * 128], DT)]
    t_ST = [din("ST0", [128, TOT[0] * 128], DT),
            din("ST1", [128, TOT[1] * 128], DT)]
    t_attr = [din("attr0", [1, TOT[0] * 128], DT),
              din("attr1", [1, TOT[1] * 128], DT)]
    t_IndT = din("IndT", [NPAD, G], DT)
    t_clinT = din("clinT", [NCLIN, G], F32)
    t_Wc_h = din("Wc_h", [H, NCLS], F32)
    t_Wc_c = din("Wc_c", [NCLIN, NCLS], F32)
    t_bc = din("bc_rep", [G, NCLS], F32)

    t_out = nc.dram_tensor("out", [G, NCLS], F32, kind="ExternalOutput").ap()

    qv_lo = nc.dram_tensor("qv_lo", [SPLIT, 2 * H], DT).ap()
    qv_hi = nc.dram_tensor("qv_hi", [NTOT - SPLIT, 2 * H], DT).ap()
    h_loc = [nc.dram_tensor(f"h_loc{l}", [128, NPAD], DT).ap() for l in range(2)]
    ag_out = [nc.dram_tensor(f"ag_out{l}", [R * 128, NPAD], DT,
                             addr_space="Shared").ap() for l in range(2)]
    pool_in = nc.dram_tensor("pool_in", [G, H], F32).ap()
    pool_out = nc.dram_tensor("pool_out", [R * G, H], F32, addr_space="Shared").ap()

    chunks = []
    c0 = 0
    while c0 < NPAD:
        csz = min(512, NPAD - c0)
        chunks.append((c0, csz))
        c0 += csz

    with tile.TileContext(nc) as tc:
        import contextlib
        with contextlib.ExitStack() as ctx:
            consts = ctx.enter_context(tc.tile_pool(name="consts", bufs=1))
            hsb = ctx.enter_context(tc.tile_pool(name="hsb", bufs=1))
            h3p = ctx.enter_context(tc.tile_pool(name="h3p", bufs=1))
            ksb = ctx.enter_context(tc.tile_pool(name="ksb", bufs=1))
            lhp = ctx.enter_context(tc.tile_pool(name="lhp", bufs=4))
            stg = ctx.enter_context(tc.tile_pool(name="stg", bufs=4))
            edg = ctx.enter_context(tc.tile_pool(name="edg", bufs=3))
            sel = ctx.enter_context(tc.tile_pool(name="sel", bufs=3))
            edm = ctx.enter_context(tc.tile_pool(name="edm", bufs=3))
            pnode = ctx.enter_context(tc.tile_pool(name="pnode", bufs=2, space="PSUM"))
            pkq = ctx.enter_context(tc.tile_pool(name="pkq", bufs=2, space="PSUM"))
            pedge = ctx.enter_context(tc.tile_pool(name="pedge", bufs=2, space="PSUM"))
            ppool = ctx.enter_context(tc.tile_pool(name="ppool", bufs=1, space="PSUM"))

            _cid = [0]

            def load_const(src_ap, shape, dt):
                _cid[0] += 1
                t = consts.tile(shape, dt, tag=f"c{_cid[0]}_{src_ap.tensor.name}")
                nc.sync.dma_start(t[:], src_ap)
                return t

            W_t = {}
            for nm, tt in (("k", t_Wk), ("q", t_Wq), ("v", t_Wv), ("s", t_Ws)):
                for l in range(NLAYER):
                    W_t[nm, l] = load_const(tt[l], [H, H], DT)
            We_t = [load_const(t_We_row[l], [1, H], DT) for l in range(NLAYER)]
            bias_k_t = [load_const(t_bias_k[l], [128, H], F32)
                        for l in range(NLAYER)] if meta["has_bias_k"] else None
            bias_qv_t = [load_const(t_bias_qv[l], [128, 2 * H], F32)
                         for l in range(NLAYER)] if meta["has_bias_qv"] else None
            bs_t = [load_const(t_bs[l], [H, 1], F32) for l in range(NLAYER)]
            A_t = [load_const(t_A[l], [H, 1], F32) for l in range(NLAYER)]
            B_t = [load_const(t_B[l], [H, 1], F32) for l in range(NLAYER)]
            ident = consts.tile([128, 128], DT)
            make_identity(nc, ident[:])
            if use_bf16:
                identf = consts.tile([128, 128], F32)
                make_identity(nc, identf[:])
            else:
                identf = ident
            gidx_t = [load_const(t_gidx[0], [128, TOT[0] * 8], I16),
                      load_const(t_gidx[1], [128, TOT[1] * 8], I16)]
            clin_t = load_const(t_clinT, [NCLIN, G], F32)
            Wch_t = load_const(t_Wc_h, [H, NCLS], F32)
            Wcc_t = load_const(t_Wc_c, [NCLIN, NCLS], F32)
            bc_t = load_const(t_bc, [G, NCLS], F32) if meta["has_bc"] else None

            hs = hsb.tile([128, NPAD], F32)        # s + agg accumulator
            k_sb = ksb.tile([128, NGRP, H], DT)    # local k table (node-major)
            h3_prev = None
            h3f = None

            # qv sub-block order: hi rows (>= SPLIT) first, then lo rows
            qv_blocks_hi, qv_blocks_lo = [], []
            for rb in range(R):
                for (c0, csz) in chunks:
                    for s in range(csz // 128):
                        row = rb * NPAD + c0 + s * 128
                        (qv_blocks_hi if row >= SPLIT else qv_blocks_lo).append(
                            (rb, c0 + s * 128))

            def emit_qv(blocks, l, hsrc):
                # group runs of 4 blocks sharing one lh load (512 cols)
                i = 0
                while i < len(blocks):
                    rb0, r0 = blocks[i]
                    run = [(rb0, r0)]
                    while (len(run) < 4 and i + len(run) < len(blocks)):
                        rbn, rn = blocks[i + len(run)]
                        if rbn == rb0 and rn == run[-1][1] + 128:
                            run.append((rbn, rn))
                        else:
                            break
                    i += len(run)
                    csz = 128 * len(run)
                    lh = lhp.tile([128, 512], DT, tag="lh")
                    nc.sync.dma_start(
                        lh[:, 0:csz], hsrc[rb0 * 128:(rb0 + 1) * 128,
                                           r0:r0 + csz])
                    for s in range(len(run)):
                        row = rb0 * NPAD + r0 + s * 128
                        ps_full = pnode.tile([128, 512], F32, tag="pn")
                        ps = ps_full[:, 0:2 * H]
                        for jj, nm in enumerate(("q", "v")):
                            nc.tensor.matmul(
                                out=ps[:, jj * H:(jj + 1) * H],
                                lhsT=lh[:, s * 128:(s + 1) * 128],
                                rhs=W_t[nm, l][:], start=True, stop=True)
                        st = stg.tile([128, 2 * H], DT, tag="st")
                        if meta["has_bias_qv"]:
                            nc.vector.tensor_tensor(
                                out=st[:], in0=ps[:], in1=bias_qv_t[l][:],
                                op=OP.add)
                        else:
                            nc.scalar.activation(st[:], ps[:], AF.Copy)
                        if row < SPLIT:
                            nc.sync.dma_start(qv_lo[row:row + 128, :], st[:])
                        else:
                            nc.sync.dma_start(
                                qv_hi[row - SPLIT:row - SPLIT + 128, :], st[:])

            def emit_edges(ep, l):
                nt_list = NT_L[ep]
                off = OFF_L[ep]
                qv_tab = qv_lo if ep == 0 else qv_hi
                for g in range(NGRP):
                    nt = int(nt_list[g])
                    if nt == 0:
                        continue
                    ne = nt * 128
                    o = int(off[g])
                    gt = edg.tile([128, TMAX, 2 * H], DT, tag="g")
                    nc.gpsimd.dma_gather(
                        gt[:, 0:nt, :], qv_tab[:], gidx_t[ep][:, o * 8:(o + nt) * 8],
                        ne, ne, 2 * H, single_packet=(ne <= 512))
                    St = sel.tile([128, TMAX, 128], DT, tag="S")
                    nc.sync.dma_start(
                        St[:, 0:nt, :],
                        t_S[ep][:, o * 128:(o + nt) * 128].rearrange(
                            "p (t d) -> p t d", t=nt))
                    STt = sel.tile([128, TMAX, 128], DT, tag="ST")
                    nc.sync.dma_start(
                        STt[:, 0:nt, :],
                        t_ST[ep][:, o * 128:(o + nt) * 128].rearrange(
                            "p (t d) -> p t d", t=nt))
                    at = sel.tile([1, TMAX * 128], DT, tag="attr")
                    nc.sync.dma_start(at[:, 0:ne],
                                      t_attr[ep][:, o * 128:o * 128 + ne])
                    ktb = edm.tile([128, TMAX, 128], DT, tag="kt")
                    for q0 in range(0, nt, KQC):
                        qn = min(KQC, nt - q0)
                        pk = pkq.tile([128, KQC, 128], F32, tag="kq")
                        for t in range(q0, q0 + qn):
                            nc.tensor.matmul(
                                out=pk[:, t - q0, :], lhsT=STt[:, t, :],
                                rhs=k_sb[:, g, :], start=True, stop=False)
                            nc.tensor.matmul(
                                out=pk[:, t - q0, :],
                                lhsT=at[0:1, t * 128:(t + 1) * 128],
                                rhs=We_t[l][:], start=False, stop=True)
                        nc.vector.tensor_tensor(
                            out=ktb[:, q0:q0 + qn, :], in0=pk[:, 0:qn, :],
                            in1=gt[:, q0:q0 + qn, 0:H], op=OP.add)
                    nc.scalar.activation(ktb[:, 0:nt, :], ktb[:, 0:nt, :],
                                         AF.Sigmoid)
                    msg = edm.tile([128, TMAX, 128], DT, tag="msg")
                    nc.vector.tensor_tensor(out=msg[:, 0:nt, :],
                                            in0=ktb[:, 0:nt, :],
                                            in1=gt[:, 0:nt, H:2 * H], op=OP.mult)
                    pa = pedge.tile([128, 128], F32, tag="pa")
                    for t in range(nt):
                        nc.tensor.matmul(out=pa[:], lhsT=msg[:, t, :],
                                         rhs=St[:, t, :], start=(t == 0),
                                         stop=(t == nt - 1))
                    nc.vector.tensor_tensor(
                        out=hs[:, g * 128:(g + 1) * 128],
                        in0=hs[:, g * 128:(g + 1) * 128], in1=pa[:], op=OP.add)

            for l in range(NLAYER):
                hsrc = t_x_rb if l == 0 else ag_out[l - 1]

                # ---- s-table (feature-major) into hs + local k table in SBUF
                for (c0, csz) in chunks:
                    if l == 0:
                        rhs_t = lhp.tile([128, 512], DT, tag="lh")
                        nc.sync.dma_start(rhs_t[:, 0:csz], t_xT_loc[:, c0:c0 + csz])
                        rhs_ap = rhs_t[:, 0:csz]
                    else:
                        rhs_ap = h3_prev[:, c0:c0 + csz]
                    pss = pnode.tile([128, 512], F32, tag="pn")
                    nc.tensor.matmul(out=pss[:, 0:csz], lhsT=W_t["s", l][:],
                                     rhs=rhs_ap, start=True, stop=True)
                    if meta["has_bs"]:
                        nc.scalar.activation(hs[:, c0:c0 + csz], pss[:, 0:csz],
                                             AF.Identity, bias=bs_t[l][:], scale=1.0)
                    else:
                        nc.scalar.activation(hs[:, c0:c0 + csz], pss[:, 0:csz],
                                             AF.Copy)
                    for s in range(csz // 128):
                        psk_full = pnode.tile([128, 512], F32, tag="pn")
                        psk = psk_full[:, 0:2 * H]
                        nc.tensor.matmul(out=psk[:, 0:H],
                                         lhsT=rhs_ap[:, s * 128:(s + 1) * 128],
                                         rhs=W_t["k", l][:], start=True, stop=True)
                        gi = c0 // 128 + s
                        if meta["has_bias_k"]:
                            nc.vector.tensor_tensor(out=k_sb[:, gi, :],
                                                    in0=psk[:, 0:H],
                                                    in1=bias_k_t[l][:], op=OP.add)
                        else:
                            nc.scalar.activation(k_sb[:, gi, :], psk[:, 0:H],
                                                 AF.Copy)

                # ---- qv node tables: hi rows first, then overlap ep1 gathers
                emit_qv(qv_blocks_hi, l, hsrc)
                if parts >= 2:
                    emit_edges(1, l)
                emit_qv(qv_blocks_lo, l, hsrc)
                if parts >= 2:
                    emit_edges(0, l)

                # ---- h update: leaky + BN
                if parts < 3:
                    break
                LCH = 896
                for lc0 in range(0, NPAD, LCH):
                    lsz = min(LCH, NPAD - lc0)
                    sl = slice(lc0, lc0 + lsz)
                    tmp = stg.tile([128, LCH], F32, tag="lrelu")
                    nc.vector.tensor_scalar_mul(tmp[:, 0:lsz], hs[:, sl], SLOPE)
                    nc.vector.tensor_tensor(out=hs[:, sl], in0=hs[:, sl],
                                            in1=tmp[:, 0:lsz], op=OP.max)
                if l < 2:
                    h3 = h3p.tile([128, NPAD], DT)
                    nc.scalar.activation(h3[:], hs[:], AF.Identity,
                                         bias=B_t[l][:], scale=A_t[l][:])
                    nc.sync.dma_start(h_loc[l][:], h3[:])
                    nc.gpsimd.collective_compute(
                        "AllGather", OP.bypass,
                        replica_groups=[list(range(R))],
                        ins=[h_loc[l][:]], outs=[ag_out[l][:]])
                    h3_prev = h3
                else:
                    h3f = hsb.tile([128, NPAD], DT)
                    nc.scalar.activation(h3f[:], hs[:], AF.Identity,
                                         bias=B_t[l][:], scale=A_t[l][:])

            if parts < 4:
                z_dbg = stg.tile([G, NCLS], F32, tag="zsb")
                nc.vector.tensor_copy(z_dbg[:], hs[0:G, 0:NCLS])
                nc.sync.dma_start(t_out[:], z_dbg[:])
            else:
                # ---- pooling
                pp = ppool.tile([G, H], F32)
                for c in range(NGRP):
                    trp = pedge.tile([128, 128], DT, tag="pa")
                    nc.tensor.transpose(out=trp[:], in_=h3f[:, c * 128:(c + 1) * 128],
                                        identity=ident[:])
                    hnode = stg.tile([128, 128], DT, tag="hnode")
                    nc.vector.tensor_copy(hnode[:], trp[:])
                    ind_t = stg.tile([128, G], DT, tag="ind")
                    nc.sync.dma_start(ind_t[:], t_IndT[c * 128:(c + 1) * 128, :])
                    nc.tensor.matmul(out=pp[:], lhsT=ind_t[:], rhs=hnode[:],
                                     start=(c == 0), stop=(c == NGRP - 1))
                pool_sb = stg.tile([G, H], F32, tag="poolsb")
                nc.vector.tensor_copy(pool_sb[:], pp[:])
                nc.sync.dma_start(pool_in[:], pool_sb[:])
                nc.gpsimd.collective_compute(
                    "AllGather", OP.bypass, replica_groups=[list(range(R))],
                    ins=[pool_in[:]], outs=[pool_out[:]])
                pr = stg.tile([G, R, H], F32, tag="pr")
                nc.sync.dma_start(pr[:], pool_out[:].rearrange("(r g) h -> g r h", r=R))
                pooled = stg.tile([G, H], F32, tag="pooled")
                nc.vector.tensor_tensor(out=pooled[:], in0=pr[:, 0, :], in1=pr[:, 1, :],
                                        op=OP.add)
                for r in range(2, R):
                    nc.vector.tensor_tensor(out=pooled[:], in0=pooled[:],
                                            in1=pr[:, r, :], op=OP.add)
                ptp = pedge.tile([H, G], F32, tag="pa")
                nc.tensor.transpose(out=ptp[:], in_=pooled[:], identity=identf[0:G, 0:G])
                pooledT = stg.tile([H, G], F32, tag="pooledT")
                nc.vector.tensor_copy(pooledT[:], ptp[:])
                zp = pedge.tile([G, NCLS], F32, tag="pa")
                nc.tensor.matmul(out=zp[:], lhsT=pooledT[:], rhs=Wch_t[:],
                                 start=True, stop=False)
                nc.tensor.matmul(out=zp[:], lhsT=clin_t[:], rhs=Wcc_t[:],
                                 start=False, stop=True)
                z_sb = stg.tile([G, NCLS], F32, tag="zsb")
                if meta["has_bc"]:
                    nc.vector.tensor_tensor(out=z_sb[:], in0=zp[:], in1=bc_t[:],
                                            op=OP.add)
                else:
                    nc.vector.tensor_copy(z_sb[:], zp[:])
                nc.sync.dma_start(t_out[:], z_sb[:])

    nc.compile()
    return nc


# ---------------------------------------------------------------------------

_CACHE = {}


def kernel(**inputs):
    in_maps, meta = prep(inputs)
    key = tuple(sorted((k, v) for k, v in meta.items()))
    if key not in _CACHE:
        _CACHE[key] = build(meta)
    nc = _CACHE[key]
    res = run_bass_kernel_spmd(nc, in_maps, list(range(R)))
    return np.asarray(res.results[0]["out"], np.float32)


def kernel_profiled(**inputs):
    """Like kernel() but also returns (exec_time_ns, trace_path)."""
    in_maps, meta = prep(inputs)
    key = tuple(sorted((k, v) for k, v in meta.items()))
    if key not in _CACHE:
        _CACHE[key] = build(meta)
    nc = _CACHE[key]
    res = run_bass_kernel_spmd(nc, in_maps, list(range(R)), trace=True)
    out = np.asarray(res.results[0]["out"], np.float32)
    trace_path = None
    if res.instructions_and_trace is not None:
        trace_path = res.instructions_and_trace[1]
    return out, res.exec_time_ns, trace_path


if __name__ == "__main__":
    pass


# revision 7
# speedup vs baseline: 1.1337x; 1.1337x over previous
"""Trainium2 Bass kernel for a 3-layer ResGatedGraphConv GNN (ClinicalGatedGCN).

Strategy (8 NeuronCores, SPMD), v2:
  - Nodes partitioned into 8 contiguous ranges (rank-blocked, padded to 128).
    Edges assigned to the rank owning their dst node, sorted by
    (rank, src-epoch, dst-group, dst) on the host.
  - Each rank computes the full [q|v] node table into local HBM (split at row
    32768 so int16 gather indices reach every row in two epochs). One
    dma_gather per (epoch, dst-group) fetches the src rows of [q|v] — this is
    the ONLY GpSimd gather; k[dst] is expanded on the PE via a host-shipped
    0/1 dst-selector S_T (matmul S_T.T @ k_group), with attr*We folded into
    the same PSUM accumulation via a K=1 rank-1 matmul.
  - The segment-sum over dst is a PE matmul against the host-shipped selector
    S (edge-major). S/S_T depend only on the edge structure, so they are
    built once on the host and streamed from DRAM each layer.
  - Gather sizes are exact per (epoch, group): nt = ceil(max-over-ranks
    count / 128) tiles, carried in meta (shapes shared across the SPMD
    program).
  - The qv table is built hi-rows-first each layer and epoch-1 gathers are
    issued before the lo rows are written, so Q7 descriptor generation
    overlaps table construction.
  - h stays feature-major; per layer the updated h slice is AllGather'd.
    Mean-pool per graph is a matmul against a host-built indicator with
    1/cnt folded in; partial pools are AllGather'd and summed; the tiny
    classifier runs on every core.
"""

import numpy as np
import ml_dtypes

import concourse.bacc as bacc
import concourse.bass as bass
import concourse.mybir as mybir
import concourse.tile as tile
from concourse.bass_utils import run_bass_kernel_spmd
from concourse.masks import make_identity

F32 = mybir.dt.float32
BF16 = mybir.dt.bfloat16
I16 = mybir.dt.int16
AF = mybir.ActivationFunctionType
OP = mybir.AluOpType

# ---------------- problem constants (hardcoded per spec) ----------------
N, E, H, G, NCLIN, NCLS = 50000, 800000, 128, 64, 16, 2
NLAYER = 3
EPS = 1e-5
SLOPE = 0.01
R = 8                      # ranks / NeuronCores
SPLIT = 32768              # int16 gather index limit -> 2 epochs

USE_BF16 = True            # table/h/gate dtype

NPR = (N + R - 1) // R     # real nodes per rank
NGRP = (NPR + 127) // 128  # 128-node groups per rank
NPAD = NGRP * 128          # padded nodes per rank
NTOT = R * NPAD            # rank-blocked total rows


def _np_dtab(use_bf16):
    return ml_dtypes.bfloat16 if use_bf16 else np.float32


def wrap_idxs_block(idx):
    """Wrap one gather call's indices: idx j -> [j%16, j//16], tiled to 128 parts."""
    n = len(idx)
    assert n % 16 == 0
    w = np.asarray(idx, np.int16).reshape(n // 16, 16).T
    return np.tile(w, (8, 1))


# ---------------------------------------------------------------------------
# host-side preprocessing
# ---------------------------------------------------------------------------

def prep(inputs, use_bf16=None):
    if use_bf16 is None:
        use_bf16 = USE_BF16
    dtab = _np_dtab(use_bf16)
    x = np.asarray(inputs["x"], np.float32)
    edge_index = np.asarray(inputs["edge_index"])
    edge_attr = np.asarray(inputs["edge_attr"], np.float32)[:, 0]
    batch = np.asarray(inputs["batch"]).astype(np.int64)
    clinical = np.asarray(inputs["clinical"], np.float32)
    Wk, bk = np.asarray(inputs["Wk"], np.float32), np.asarray(inputs["bk"], np.float32)
    Wq, bq = np.asarray(inputs["Wq"], np.float32), np.asarray(inputs["bq"], np.float32)
    Wv, bv = np.asarray(inputs["Wv"], np.float32), np.asarray(inputs["bv"], np.float32)
    Ws, bs = np.asarray(inputs["Ws"], np.float32), np.asarray(inputs["bs"], np.float32)
    We, be = np.asarray(inputs["We"], np.float32), np.asarray(inputs["be"], np.float32)
    gamma = np.asarray(inputs["gamma"], np.float32)
    beta = np.asarray(inputs["beta"], np.float32)
    rmean = np.asarray(inputs["rmean"], np.float32)
    rvar = np.asarray(inputs["rvar"], np.float32)
    Wc, bc = np.asarray(inputs["Wc"], np.float32), np.asarray(inputs["bc"], np.float32)

    src = edge_index[0].astype(np.int64)
    dst = edge_index[1].astype(np.int64)

    # BN folded: A*x + B
    A = gamma / np.sqrt(rvar + EPS)
    B = beta - rmean * A
    bgate = bk + bq + be          # folded into k table bias
    rb_row = (src // NPR) * NPAD + (src % NPR)

    e_rank = dst // NPR
    epoch = (rb_row >= SPLIT).astype(np.int64)
    dst_local = dst - e_rank * NPR
    group = dst_local // 128
    dst_rel = dst_local % 128

    # per (ep, rank, group) counts -> exact per-(ep,group) tile counts
    counts = np.zeros((2, R, NGRP), np.int64)
    np.add.at(counts, (epoch, e_rank, group), 1)
    nt_l = [np.ceil(counts[ep].max(axis=0) / 128).astype(int) for ep in (0, 1)]
    off_l = [np.concatenate([[0], np.cumsum(nt)]).astype(int) for nt in nt_l]

    # graph counts for mean pooling
    cntg = np.bincount(batch, minlength=G).astype(np.float32)
    inv_cnt = 1.0 / np.maximum(cntg, 1.0)

    order = np.lexsort((dst, group, epoch, e_rank))
    src_s, dst_rel_s, attr_s = rb_row[order], dst_rel[order], edge_attr[order]
    ep_s, rank_s, grp_s = epoch[order], e_rank[order], group[order]
    key = ((rank_s * 2 + ep_s) * NGRP + grp_s)
    starts = np.searchsorted(key, np.arange(R * 2 * NGRP + 1))

    bias_k = np.zeros((NLAYER, 128, H), np.float32)
    for l in range(NLAYER):
        bias_k[l, :, :] = bgate[l][None, :]
    bias_v = np.zeros((NLAYER, 128, 2 * H), np.float32)
    for l in range(NLAYER):
        bias_v[l, :, H:2 * H] = bv[l][None, :]
    has_bias_k = bool(np.any(bias_k != 0))
    has_bias_qv = bool(np.any(bias_v != 0))
    has_bs = bool(np.any(bs != 0))
    has_bc = bool(np.any(bc != 0))
    We_row = We[:, 0, :].reshape(NLAYER, 1, H)

    x_rb = np.zeros((R * 128, NPAD), np.float32)
    for r in range(R):
        lo, hi = r * NPR, min((r + 1) * NPR, N)
        x_rb[r * 128:(r + 1) * 128, 0:hi - lo] = x[lo:hi].T

    in_maps = []
    for r in range(R):
        ep_arrs = {}
        for ep in (0, 1):
            nt = nt_l[ep]
            off = off_l[ep]
            tot_tiles = int(off[-1])
            slots = tot_tiles * 128
            gidx = np.zeros((128, tot_tiles * 8), np.int16)
            S = np.zeros((128, slots), dtab)
            ST = np.zeros((128, slots), dtab)
            attr_row = np.zeros((1, slots), dtab)
            for g in range(NGRP):
                ntg = int(nt[g])
                if ntg == 0:
                    continue
                k = (r * 2 + ep) * NGRP + g
                s0 = int(starts[k])
                n = int(counts[ep, r, g])
                o = int(off[g])
                idx = np.zeros((ntg * 128,), np.int64)
                idx[:n] = src_s[s0:s0 + n] - ep * SPLIT
                gidx[:, o * 8:(o + ntg) * 8] = wrap_idxs_block(idx)
                j = np.arange(n)
                t = j // 128
                p = j % 128
                drel = dst_rel_s[s0:s0 + n].astype(np.int64)
                col = (o + t) * 128
                S[p, col + drel] = 1
                ST[drel, col + p] = 1
                attr_row[0, o * 128 + j] = attr_s[s0:s0 + n].astype(dtab)
            ep_arrs[ep] = (gidx, S, ST, attr_row)
        IndT = np.zeros((NPAD, G), np.float32)
        lo, hi = r * NPR, min((r + 1) * NPR, N)
        IndT[np.arange(hi - lo), batch[lo:hi]] = inv_cnt[batch[lo:hi]]
        im = {
            "x_rb": x_rb.astype(dtab),
            "xT_loc": x_rb[r * 128:(r + 1) * 128].astype(dtab),
            "Wk": Wk.astype(dtab), "Wq": Wq.astype(dtab), "Wv": Wv.astype(dtab),
            "Ws": Ws.astype(dtab),
            "We_row": We_row.astype(dtab),
            "bias_k": bias_k,
            "bias_qv": bias_v,
            "bs_col": bs.reshape(NLAYER, H, 1),
            "A_col": A.reshape(NLAYER, H, 1),
            "B_col": B.reshape(NLAYER, H, 1),
            "gidx0": ep_arrs[0][0], "S0": ep_arrs[0][1], "ST0": ep_arrs[0][2],
            "attr0": ep_arrs[0][3],
            "gidx1": ep_arrs[1][0], "S1": ep_arrs[1][1], "ST1": ep_arrs[1][2],
            "attr1": ep_arrs[1][3],
            "IndT": IndT.astype(dtab),
            "clinT": clinical.T.copy(),
            "Wc_h": Wc[0:H], "Wc_c": Wc[H:H + NCLIN],
            "bc_rep": np.tile(bc, (G, 1)),
        }
        in_maps.append(im)
    meta = dict(NT0=tuple(int(v) for v in nt_l[0]),
                NT1=tuple(int(v) for v in nt_l[1]),
                has_bias_k=has_bias_k, has_bias_qv=has_bias_qv,
                has_bs=has_bs, has_bc=has_bc, use_bf16=use_bf16)
    return in_maps, meta


# ---------------------------------------------------------------------------
# device program
# ---------------------------------------------------------------------------

def build(meta):
    use_bf16 = meta["use_bf16"]
    parts = meta.get("parts", 4)
    DT = BF16 if use_bf16 else F32
    NT_L = [list(meta["NT0"]), list(meta["NT1"])]
    OFF_L = [np.concatenate([[0], np.cumsum(nt)]).astype(int) for nt in NT_L]
    TOT = [int(o[-1]) for o in OFF_L]
    TMAX = max(max(NT_L[0]), max(NT_L[1]))
    KQC = 4                           # kq psum chunk (tiles, 1 PSUM bank)

    nc = bacc.Bacc("TRN2", target_bir_lowering=False, debug=False, num_devices=R)

    def din(name, shape, dt):
        return nc.dram_tensor(name, shape, dt, kind="ExternalInput").ap()

    t_x_rb = din("x_rb", [R * 128, NPAD], DT)
    t_xT_loc = din("xT_loc", [128, NPAD], DT)
    t_Wk = din("Wk", [NLAYER, H, H], DT)
    t_Wq = din("Wq", [NLAYER, H, H], DT)
    t_Wv = din("Wv", [NLAYER, H, H], DT)
    t_Ws = din("Ws", [NLAYER, H, H], DT)
    t_We_row = din("We_row", [NLAYER, 1, H], DT)
    t_bias_k = din("bias_k", [NLAYER, 128, H], F32)
    t_bias_qv = din("bias_qv", [NLAYER, 128, 2 * H], F32)
    t_bs = din("bs_col", [NLAYER, H, 1], F32)
    t_A = din("A_col", [NLAYER, H, 1], F32)
    t_B = din("B_col", [NLAYER, H, 1], F32)
    t_gidx = [din("gidx0", [128, TOT[0] * 8], I16),
              din("gidx1", [128, TOT[1] * 8], I16)]
    t_S = [din("S0", [128, TOT[0] * 128], DT),
           din("S1", [128, TOT[1] * 128], DT)]
    t_ST = [din("ST0", [128, TOT[0] * 128], DT),
            din("ST1", [128, TOT[1] * 128], DT)]
    t_attr = [din("attr0", [1, TOT[0] * 128], DT),
              din("attr1", [1, TOT[1] * 128], DT)]
    t_IndT = din("IndT", [NPAD, G], DT)
    t_clinT = din("clinT", [NCLIN, G], F32)
    t_Wc_h = din("Wc_h", [H, NCLS], F32)
    t_Wc_c = din("Wc_c", [NCLIN, NCLS], F32)
    t_bc = din("bc_rep", [G, NCLS], F32)

    t_out = nc.dram_tensor("out", [G, NCLS], F32, kind="ExternalOutput").ap()

    qv_lo = nc.dram_tensor("qv_lo", [SPLIT, 2 * H], DT).ap()
    qv_hi = nc.dram_tensor("qv_hi", [NTOT - SPLIT, 2 * H], DT).ap()
    h_loc = [nc.dram_tensor(f"h_loc{l}", [128, NPAD], DT).ap() for l in range(2)]
    ag_out = [nc.dram_tensor(f"ag_out{l}", [R * 128, NPAD], DT,
                             addr_space="Shared").ap() for l in range(2)]
    pool_in = nc.dram_tensor("pool_in", [G, H], F32).ap()
    pool_out = nc.dram_tensor("pool_out", [R * G, H], F32, addr_space="Shared").ap()

    chunks = []
    c0 = 0
    while c0 < NPAD:
        csz = min(512, NPAD - c0)
        chunks.append((c0, csz))
        c0 += csz

    with tile.TileContext(nc) as tc:
        import contextlib
        with contextlib.ExitStack() as ctx:
            consts = ctx.enter_context(tc.tile_pool(name="consts", bufs=1))
            hsb = ctx.enter_context(tc.tile_pool(name="hsb", bufs=1))
            h3p = ctx.enter_context(tc.tile_pool(name="h3p", bufs=1))
            ksb = ctx.enter_context(tc.tile_pool(name="ksb", bufs=1))
            lhp = ctx.enter_context(tc.tile_pool(name="lhp", bufs=4))
            stg = ctx.enter_context(tc.tile_pool(name="stg", bufs=4))
            edg = ctx.enter_context(tc.tile_pool(name="edg", bufs=3))
            sel = ctx.enter_context(tc.tile_pool(name="sel", bufs=3))
            edm = ctx.enter_context(tc.tile_pool(name="edm", bufs=3))
            pnode = ctx.enter_context(tc.tile_pool(name="pnode", bufs=2, space="PSUM"))
            pkq = ctx.enter_context(tc.tile_pool(name="pkq", bufs=2, space="PSUM"))
            pedge = ctx.enter_context(tc.tile_pool(name="pedge", bufs=2, space="PSUM"))
            ppool = ctx.enter_context(tc.tile_pool(name="ppool", bufs=1, space="PSUM"))

            _cid = [0]

            def load_const(src_ap, shape, dt):
                _cid[0] += 1
                t = consts.tile(shape, dt, tag=f"c{_cid[0]}_{src_ap.tensor.name}")
                nc.sync.dma_start(t[:], src_ap)
                return t

            W_t = {}
            for nm, tt in (("k", t_Wk), ("q", t_Wq), ("v", t_Wv), ("s", t_Ws)):
                for l in range(NLAYER):
                    W_t[nm, l] = load_const(tt[l], [H, H], DT)
            We_t = [load_const(t_We_row[l], [1, H], DT) for l in range(NLAYER)]
            bias_k_t = [load_const(t_bias_k[l], [128, H], F32)
                        for l in range(NLAYER)] if meta["has_bias_k"] else None
            bias_qv_t = [load_const(t_bias_qv[l], [128, 2 * H], F32)
                         for l in range(NLAYER)] if meta["has_bias_qv"] else None
            bs_t = [load_const(t_bs[l], [H, 1], F32) for l in range(NLAYER)]
            A_t = [load_const(t_A[l], [H, 1], F32) for l in range(NLAYER)]
            B_t = [load_const(t_B[l], [H, 1], F32) for l in range(NLAYER)]
            ident = consts.tile([128, 128], DT)
            make_identity(nc, ident[:])
            if use_bf16:
                identf = consts.tile([128, 128], F32)
                make_identity(nc, identf[:])
            else:
                identf = ident
            gidx_t = [load_const(t_gidx[0], [128, TOT[0] * 8], I16),
                      load_const(t_gidx[1], [128, TOT[1] * 8], I16)]
            clin_t = load_const(t_clinT, [NCLIN, G], F32)
            Wch_t = load_const(t_Wc_h, [H, NCLS], F32)
            Wcc_t = load_const(t_Wc_c, [NCLIN, NCLS], F32)
            bc_t = load_const(t_bc, [G, NCLS], F32) if meta["has_bc"] else None

            hs = hsb.tile([128, NPAD], F32)        # s + agg accumulator
            k_sb = ksb.tile([128, NGRP, H], DT)    # local k table (node-major)
            h3_prev = None
            h3f = None

            # qv sub-block order: hi rows (>= SPLIT) first, then lo rows
            qv_blocks_hi, qv_blocks_lo = [], []
            for rb in range(R):
                for (c0, csz) in chunks:
                    for s in range(csz // 128):
                        row = rb * NPAD + c0 + s * 128
                        (qv_blocks_hi if row >= SPLIT else qv_blocks_lo).append(
                            (rb, c0 + s * 128))

            def make_runs(blocks):
                # group runs of 4 blocks sharing one lh load (512 cols)
                runs = []
                i = 0
                while i < len(blocks):
                    rb0, r0 = blocks[i]
                    run = [(rb0, r0)]
                    while (len(run) < 4 and i + len(run) < len(blocks)):
                        rbn, rn = blocks[i + len(run)]
                        if rbn == rb0 and rn == run[-1][1] + 128:
                            run.append((rbn, rn))
                        else:
                            break
                    i += len(run)
                    runs.append((rb0, r0, len(run)))
                return runs

            def emit_run(rb0, r0, nrun, l, hsrc):
                    csz = 128 * nrun
                    lh = lhp.tile([128, 512], DT, tag="lh")
                    nc.sync.dma_start(
                        lh[:, 0:csz], hsrc[rb0 * 128:(rb0 + 1) * 128,
                                           r0:r0 + csz])
                    for s in range(nrun):
                        row = rb0 * NPAD + r0 + s * 128
                        ps_full = pnode.tile([128, 512], F32, tag="pn")
                        ps = ps_full[:, 0:2 * H]
                        for jj, nm in enumerate(("q", "v")):
                            nc.tensor.matmul(
                                out=ps[:, jj * H:(jj + 1) * H],
                                lhsT=lh[:, s * 128:(s + 1) * 128],
                                rhs=W_t[nm, l][:], start=True, stop=True)
                        st = stg.tile([128, 2 * H], DT, tag="st")
                        if meta["has_bias_qv"]:
                            nc.vector.tensor_tensor(
                                out=st[:], in0=ps[:], in1=bias_qv_t[l][:],
                                op=OP.add)
                        else:
                            nc.scalar.activation(st[:], ps[:], AF.Copy)
                        if row < SPLIT:
                            nc.sync.dma_start(qv_lo[row:row + 128, :], st[:])
                        else:
                            nc.sync.dma_start(
                                qv_hi[row - SPLIT:row - SPLIT + 128, :], st[:])

            def emit_qv(blocks, l, hsrc):
                for (rb0, r0, nrun) in make_runs(blocks):
                    emit_run(rb0, r0, nrun, l, hsrc)

            def emit_edges(ep, l, filler=None):
                nt_list = NT_L[ep]
                off = OFF_L[ep]
                qv_tab = qv_lo if ep == 0 else qv_hi
                for g in range(NGRP):
                    if filler is not None:
                        filler(g)
                    nt = int(nt_list[g])
                    if nt == 0:
                        continue
                    ne = nt * 128
                    o = int(off[g])
                    gt = edg.tile([128, TMAX, 2 * H], DT, tag="g")
                    nc.gpsimd.dma_gather(
                        gt[:, 0:nt, :], qv_tab[:], gidx_t[ep][:, o * 8:(o + nt) * 8],
                        ne, ne, 2 * H, single_packet=(ne <= 512))
                    St = sel.tile([128, TMAX, 128], DT, tag="S")
                    nc.sync.dma_start(
                        St[:, 0:nt, :],
                        t_S[ep][:, o * 128:(o + nt) * 128].rearrange(
                            "p (t d) -> p t d", t=nt))
                    STt = sel.tile([128, TMAX, 128], DT, tag="ST")
                    nc.sync.dma_start(
                        STt[:, 0:nt, :],
                        t_ST[ep][:, o * 128:(o + nt) * 128].rearrange(
                            "p (t d) -> p t d", t=nt))
                    at = sel.tile([1, TMAX * 128], DT, tag="attr")
                    nc.sync.dma_start(at[:, 0:ne],
                                      t_attr[ep][:, o * 128:o * 128 + ne])
                    ktb = edm.tile([128, TMAX, 128], DT, tag="kt")
                    for q0 in range(0, nt, KQC):
                        qn = min(KQC, nt - q0)
                        pk = pkq.tile([128, KQC, 128], F32, tag="kq")
                        for t in range(q0, q0 + qn):
                            nc.tensor.matmul(
                                out=pk[:, t - q0, :], lhsT=STt[:, t, :],
                                rhs=k_sb[:, g, :], start=True, stop=False)
                            nc.tensor.matmul(
                                out=pk[:, t - q0, :],
                                lhsT=at[0:1, t * 128:(t + 1) * 128],
                                rhs=We_t[l][:], start=False, stop=True)
                        nc.vector.tensor_tensor(
                            out=ktb[:, q0:q0 + qn, :], in0=pk[:, 0:qn, :],
                            in1=gt[:, q0:q0 + qn, 0:H], op=OP.add)
                    nc.scalar.activation(ktb[:, 0:nt, :], ktb[:, 0:nt, :],
                                         AF.Sigmoid)
                    msg = edm.tile([128, TMAX, 128], DT, tag="msg")
                    nc.vector.tensor_tensor(out=msg[:, 0:nt, :],
                                            in0=ktb[:, 0:nt, :],
                                            in1=gt[:, 0:nt, H:2 * H], op=OP.mult)
                    pa = pedge.tile([128, 128], F32, tag="pa")
                    for t in range(nt):
                        nc.tensor.matmul(out=pa[:], lhsT=msg[:, t, :],
                                         rhs=St[:, t, :], start=(t == 0),
                                         stop=(t == nt - 1))
                    nc.vector.tensor_tensor(
                        out=hs[:, g * 128:(g + 1) * 128],
                        in0=hs[:, g * 128:(g + 1) * 128], in1=pa[:], op=OP.add)

            for l in range(NLAYER):
                hsrc = t_x_rb if l == 0 else ag_out[l - 1]

                # ---- s-table (feature-major) into hs + local k table in SBUF
                for (c0, csz) in chunks:
                    if l == 0:
                        rhs_t = lhp.tile([128, 512], DT, tag="lh")
                        nc.sync.dma_start(rhs_t[:, 0:csz], t_xT_loc[:, c0:c0 + csz])
                        rhs_ap = rhs_t[:, 0:csz]
                    else:
                        rhs_ap = h3_prev[:, c0:c0 + csz]
                    pss = pnode.tile([128, 512], F32, tag="pn")
                    nc.tensor.matmul(out=pss[:, 0:csz], lhsT=W_t["s", l][:],
                                     rhs=rhs_ap, start=True, stop=True)
                    if meta["has_bs"]:
                        nc.scalar.activation(hs[:, c0:c0 + csz], pss[:, 0:csz],
                                             AF.Identity, bias=bs_t[l][:], scale=1.0)
                    else:
                        nc.scalar.activation(hs[:, c0:c0 + csz], pss[:, 0:csz],
                                             AF.Copy)
                    for s in range(csz // 128):
                        psk_full = pnode.tile([128, 512], F32, tag="pn")
                        psk = psk_full[:, 0:2 * H]
                        nc.tensor.matmul(out=psk[:, 0:H],
                                         lhsT=rhs_ap[:, s * 128:(s + 1) * 128],
                                         rhs=W_t["k", l][:], start=True, stop=True)
                        gi = c0 // 128 + s
                        if meta["has_bias_k"]:
                            nc.vector.tensor_tensor(out=k_sb[:, gi, :],
                                                    in0=psk[:, 0:H],
                                                    in1=bias_k_t[l][:], op=OP.add)
                        else:
                            nc.scalar.activation(k_sb[:, gi, :], psk[:, 0:H],
                                                 AF.Copy)

                # ---- qv node tables: hi rows first; lo rows interleaved
                # into the ep1 edge phase so PE/DMA build them while Q7 gathers
                emit_qv(qv_blocks_hi, l, hsrc)
                lo_runs = make_runs(qv_blocks_lo)
                pos = [0]
                _l, _hsrc = l, hsrc

                def filler(g, _l=l, _hsrc=hsrc, pos=pos, lo_runs=lo_runs):
                    tgt = (g + 1) * len(lo_runs) // NGRP
                    while pos[0] < min(tgt, len(lo_runs)):
                        rb0, r0, nrun = lo_runs[pos[0]]
                        emit_run(rb0, r0, nrun, _l, _hsrc)
                        pos[0] += 1

                if parts >= 2:
                    emit_edges(1, l, filler)
                while pos[0] < len(lo_runs):
                    rb0, r0, nrun = lo_runs[pos[0]]
                    emit_run(rb0, r0, nrun, l, hsrc)
                    pos[0] += 1
                if parts >= 2:
                    emit_edges(0, l)

                # ---- h update: leaky + BN
                if parts < 3:
                    break
                LCH = 896
                for lc0 in range(0, NPAD, LCH):
                    lsz = min(LCH, NPAD - lc0)
                    sl = slice(lc0, lc0 + lsz)
                    tmp = stg.tile([128, LCH], F32, tag="lrelu")
                    nc.vector.tensor_scalar_mul(tmp[:, 0:lsz], hs[:, sl], SLOPE)
                    nc.vector.tensor_tensor(out=hs[:, sl], in0=hs[:, sl],
                                            in1=tmp[:, 0:lsz], op=OP.max)
                if l < 2:
                    h3 = h3p.tile([128, NPAD], DT)
                    nc.scalar.activation(h3[:], hs[:], AF.Identity,
                                         bias=B_t[l][:], scale=A_t[l][:])
                    nc.sync.dma_start(h_loc[l][:], h3[:])
                    nc.gpsimd.collective_compute(
                        "AllGather", OP.bypass,
                        replica_groups=[list(range(R))],
                        ins=[h_loc[l][:]], outs=[ag_out[l][:]])
                    h3_prev = h3
                else:
                    h3f = hsb.tile([128, NPAD], DT)
                    nc.scalar.activation(h3f[:], hs[:], AF.Identity,
                                         bias=B_t[l][:], scale=A_t[l][:])

            if parts < 4:
                z_dbg = stg.tile([G, NCLS], F32, tag="zsb")
                nc.vector.tensor_copy(z_dbg[:], hs[0:G, 0:NCLS])
                nc.sync.dma_start(t_out[:], z_dbg[:])
            else:
                # ---- pooling
                pp = ppool.tile([G, H], F32)
                for c in range(NGRP):
                    trp = pedge.tile([128, 128], DT, tag="pa")
                    nc.tensor.transpose(out=trp[:], in_=h3f[:, c * 128:(c + 1) * 128],
                                        identity=ident[:])
                    hnode = stg.tile([128, 128], DT, tag="hnode")
                    nc.vector.tensor_copy(hnode[:], trp[:])
                    ind_t = stg.tile([128, G], DT, tag="ind")
                    nc.sync.dma_start(ind_t[:], t_IndT[c * 128:(c + 1) * 128, :])
                    nc.tensor.matmul(out=pp[:], lhsT=ind_t[:], rhs=hnode[:],
                                     start=(c == 0), stop=(c == NGRP - 1))
                pool_sb = stg.tile([G, H], F32, tag="poolsb")
                nc.vector.tensor_copy(pool_sb[:], pp[:])
                nc.sync.dma_start(pool_in[:], pool_sb[:])
                nc.gpsimd.collective_compute(
                    "AllGather", OP.bypass, replica_groups=[list(range(R))],
                    ins=[pool_in[:]], outs=[pool_out[:]])
                pr = stg.tile([G, R, H], F32, tag="pr")
                nc.sync.dma_start(pr[:], pool_out[:].rearrange("(r g) h -> g r h", r=R))
                pooled = stg.tile([G, H], F32, tag="pooled")
                nc.vector.tensor_tensor(out=pooled[:], in0=pr[:, 0, :], in1=pr[:, 1, :],
                                        op=OP.add)
                for r in range(2, R):
                    nc.vector.tensor_tensor(out=pooled[:], in0=pooled[:],
                                            in1=pr[:, r, :], op=OP.add)
                ptp = pedge.tile([H, G], F32, tag="pa")
                nc.tensor.transpose(out=ptp[:], in_=pooled[:], identity=identf[0:G, 0:G])
                pooledT = stg.tile([H, G], F32, tag="pooledT")
                nc.vector.tensor_copy(pooledT[:], ptp[:])
                zp = pedge.tile([G, NCLS], F32, tag="pa")
                nc.tensor.matmul(out=zp[:], lhsT=pooledT[:], rhs=Wch_t[:],
                                 start=True, stop=False)
                nc.tensor.matmul(out=zp[:], lhsT=clin_t[:], rhs=Wcc_t[:],
                                 start=False, stop=True)
                z_sb = stg.tile([G, NCLS], F32, tag="zsb")
                if meta["has_bc"]:
                    nc.vector.tensor_tensor(out=z_sb[:], in0=zp[:], in1=bc_t[:],
                                            op=OP.add)
                else:
                    nc.vector.tensor_copy(z_sb[:], zp[:])
                nc.sync.dma_start(t_out[:], z_sb[:])

    nc.compile()
    return nc


# ---------------------------------------------------------------------------

_CACHE = {}


def kernel(**inputs):
    in_maps, meta = prep(inputs)
    key = tuple(sorted((k, v) for k, v in meta.items()))
    if key not in _CACHE:
        _CACHE[key] = build(meta)
    nc = _CACHE[key]
    res = run_bass_kernel_spmd(nc, in_maps, list(range(R)))
    return np.asarray(res.results[0]["out"], np.float32)


def kernel_profiled(**inputs):
    """Like kernel() but also returns (exec_time_ns, trace_path)."""
    in_maps, meta = prep(inputs)
    key = tuple(sorted((k, v) for k, v in meta.items()))
    if key not in _CACHE:
        _CACHE[key] = build(meta)
    nc = _CACHE[key]
    res = run_bass_kernel_spmd(nc, in_maps, list(range(R)), trace=True)
    out = np.asarray(res.results[0]["out"], np.float32)
    trace_path = None
    if res.instructions_and_trace is not None:
        trace_path = res.instructions_and_trace[1]
    return out, res.exec_time_ns, trace_path


if __name__ == "__main__":
    pass


# revision 9
# speedup vs baseline: 1.1371x; 1.0030x over previous
"""Trainium2 Bass kernel for a 3-layer ResGatedGraphConv GNN (ClinicalGatedGCN).

Strategy (8 NeuronCores, SPMD), v2:
  - Nodes partitioned into 8 contiguous ranges (rank-blocked, padded to 128).
    Edges assigned to the rank owning their dst node, sorted by
    (rank, src-epoch, dst-group, dst) on the host.
  - Each rank computes the full [q|v] node table into local HBM (split at row
    32768 so int16 gather indices reach every row in two epochs). One
    dma_gather per (epoch, dst-group) fetches the src rows of [q|v] — this is
    the ONLY GpSimd gather; k[dst] is expanded on the PE via a host-shipped
    0/1 dst-selector S_T (matmul S_T.T @ k_group), with attr*We folded into
    the same PSUM accumulation via a K=1 rank-1 matmul.
  - The segment-sum over dst is a PE matmul against the host-shipped selector
    S (edge-major). S/S_T depend only on the edge structure, so they are
    built once on the host and streamed from DRAM each layer.
  - Gather sizes are exact per (epoch, group): nt = ceil(max-over-ranks
    count / 128) tiles, carried in meta (shapes shared across the SPMD
    program).
  - The qv table is built hi-rows-first each layer and epoch-1 gathers are
    issued before the lo rows are written, so Q7 descriptor generation
    overlaps table construction.
  - h stays feature-major; per layer the updated h slice is AllGather'd.
    Mean-pool per graph is a matmul against a host-built indicator with
    1/cnt folded in; partial pools are AllGather'd and summed; the tiny
    classifier runs on every core.
"""

import numpy as np
import ml_dtypes

import concourse.bacc as bacc
import concourse.bass as bass
import concourse.mybir as mybir
import concourse.tile as tile
from concourse.bass_utils import run_bass_kernel_spmd
from concourse.masks import make_identity

F32 = mybir.dt.float32
BF16 = mybir.dt.bfloat16
I16 = mybir.dt.int16
AF = mybir.ActivationFunctionType
OP = mybir.AluOpType

# ---------------- problem constants (hardcoded per spec) ----------------
N, E, H, G, NCLIN, NCLS = 50000, 800000, 128, 64, 16, 2
NLAYER = 3
EPS = 1e-5
SLOPE = 0.01
R = 8                      # ranks / NeuronCores
SPLIT = 32768              # int16 gather index limit -> 2 epochs

USE_BF16 = True            # table/h/gate dtype

NPR = (N + R - 1) // R     # real nodes per rank
NGRP = (NPR + 127) // 128  # 128-node groups per rank
NPAD = NGRP * 128          # padded nodes per rank
NTOT = R * NPAD            # rank-blocked total rows


def _np_dtab(use_bf16):
    return ml_dtypes.bfloat16 if use_bf16 else np.float32


def wrap_idxs_block(idx):
    """Wrap one gather call's indices: idx j -> [j%16, j//16], tiled to 128 parts."""
    n = len(idx)
    assert n % 16 == 0
    w = np.asarray(idx, np.int16).reshape(n // 16, 16).T
    return np.tile(w, (8, 1))


# ---------------------------------------------------------------------------
# host-side preprocessing
# ---------------------------------------------------------------------------

def prep(inputs, use_bf16=None):
    if use_bf16 is None:
        use_bf16 = USE_BF16
    dtab = _np_dtab(use_bf16)
    x = np.asarray(inputs["x"], np.float32)
    edge_index = np.asarray(inputs["edge_index"])
    edge_attr = np.asarray(inputs["edge_attr"], np.float32)[:, 0]
    batch = np.asarray(inputs["batch"]).astype(np.int64)
    clinical = np.asarray(inputs["clinical"], np.float32)
    Wk, bk = np.asarray(inputs["Wk"], np.float32), np.asarray(inputs["bk"], np.float32)
    Wq, bq = np.asarray(inputs["Wq"], np.float32), np.asarray(inputs["bq"], np.float32)
    Wv, bv = np.asarray(inputs["Wv"], np.float32), np.asarray(inputs["bv"], np.float32)
    Ws, bs = np.asarray(inputs["Ws"], np.float32), np.asarray(inputs["bs"], np.float32)
    We, be = np.asarray(inputs["We"], np.float32), np.asarray(inputs["be"], np.float32)
    gamma = np.asarray(inputs["gamma"], np.float32)
    beta = np.asarray(inputs["beta"], np.float32)
    rmean = np.asarray(inputs["rmean"], np.float32)
    rvar = np.asarray(inputs["rvar"], np.float32)
    Wc, bc = np.asarray(inputs["Wc"], np.float32), np.asarray(inputs["bc"], np.float32)

    src = edge_index[0].astype(np.int64)
    dst = edge_index[1].astype(np.int64)

    # BN folded: A*x + B
    A = gamma / np.sqrt(rvar + EPS)
    B = beta - rmean * A
    bgate = bk + bq + be          # folded into k table bias
    rb_row = (src // NPR) * NPAD + (src % NPR)

    e_rank = dst // NPR
    epoch = (rb_row >= SPLIT).astype(np.int64)
    dst_local = dst - e_rank * NPR
    group = dst_local // 128
    dst_rel = dst_local % 128

    # per (ep, rank, group) counts -> exact per-(ep,group) tile counts
    counts = np.zeros((2, R, NGRP), np.int64)
    np.add.at(counts, (epoch, e_rank, group), 1)
    nt_l = [np.ceil(counts[ep].max(axis=0) / 128).astype(int) for ep in (0, 1)]
    off_l = [np.concatenate([[0], np.cumsum(nt)]).astype(int) for nt in nt_l]

    # graph counts for mean pooling
    cntg = np.bincount(batch, minlength=G).astype(np.float32)
    inv_cnt = 1.0 / np.maximum(cntg, 1.0)

    order = np.lexsort((dst, group, epoch, e_rank))
    src_s, dst_rel_s, attr_s = rb_row[order], dst_rel[order], edge_attr[order]
    ep_s, rank_s, grp_s = epoch[order], e_rank[order], group[order]
    key = ((rank_s * 2 + ep_s) * NGRP + grp_s)
    starts = np.searchsorted(key, np.arange(R * 2 * NGRP + 1))

    bias_k = np.zeros((NLAYER, 128, H), np.float32)
    for l in range(NLAYER):
        bias_k[l, :, :] = bgate[l][None, :]
    bias_v = np.zeros((NLAYER, 128, 2 * H), np.float32)
    for l in range(NLAYER):
        bias_v[l, :, H:2 * H] = bv[l][None, :]
    has_bias_k = bool(np.any(bias_k != 0))
    has_bias_qv = bool(np.any(bias_v != 0))
    has_bs = bool(np.any(bs != 0))
    has_bc = bool(np.any(bc != 0))
    We_row = We[:, 0, :].reshape(NLAYER, 1, H)

    x_rb = np.zeros((R * 128, NPAD), np.float32)
    for r in range(R):
        lo, hi = r * NPR, min((r + 1) * NPR, N)
        x_rb[r * 128:(r + 1) * 128, 0:hi - lo] = x[lo:hi].T

    in_maps = []
    for r in range(R):
        ep_arrs = {}
        for ep in (0, 1):
            nt = nt_l[ep]
            off = off_l[ep]
            tot_tiles = int(off[-1])
            slots = tot_tiles * 128
            gidx = np.zeros((128, tot_tiles * 8), np.int16)
            S = np.zeros((128, slots), dtab)
            ST = np.zeros((128, slots), dtab)
            attr_row = np.zeros((1, slots), dtab)
            for g in range(NGRP):
                ntg = int(nt[g])
                if ntg == 0:
                    continue
                k = (r * 2 + ep) * NGRP + g
                s0 = int(starts[k])
                n = int(counts[ep, r, g])
                o = int(off[g])
                idx = np.zeros((ntg * 128,), np.int64)
                idx[:n] = src_s[s0:s0 + n] - ep * SPLIT
                gidx[:, o * 8:(o + ntg) * 8] = wrap_idxs_block(idx)
                j = np.arange(n)
                t = j // 128
                p = j % 128
                drel = dst_rel_s[s0:s0 + n].astype(np.int64)
                col = (o + t) * 128
                S[p, col + drel] = 1
                ST[drel, col + p] = 1
                attr_row[0, o * 128 + j] = attr_s[s0:s0 + n].astype(dtab)
            ep_arrs[ep] = (gidx, S, ST, attr_row)
        IndT = np.zeros((NPAD, G), np.float32)
        lo, hi = r * NPR, min((r + 1) * NPR, N)
        IndT[np.arange(hi - lo), batch[lo:hi]] = inv_cnt[batch[lo:hi]]
        im = {
            "x_rb": x_rb.astype(dtab),
            "xT_loc": x_rb[r * 128:(r + 1) * 128].astype(dtab),
            "Wk": Wk.astype(dtab), "Wq": Wq.astype(dtab), "Wv": Wv.astype(dtab),
            "Ws": Ws.astype(dtab),
            "We_row": We_row.astype(dtab),
            "bias_k": bias_k,
            "bias_qv": bias_v,
            "bs_col": bs.reshape(NLAYER, H, 1),
            "A_col": A.reshape(NLAYER, H, 1),
            "B_col": B.reshape(NLAYER, H, 1),
            "gidx0": ep_arrs[0][0], "S0": ep_arrs[0][1], "ST0": ep_arrs[0][2],
            "attr0": ep_arrs[0][3],
            "gidx1": ep_arrs[1][0], "S1": ep_arrs[1][1], "ST1": ep_arrs[1][2],
            "attr1": ep_arrs[1][3],
            "IndT": IndT.astype(dtab),
            "clinT": clinical.T.copy(),
            "Wc_h": Wc[0:H], "Wc_c": Wc[H:H + NCLIN],
            "bc_rep": np.tile(bc, (G, 1)),
        }
        in_maps.append(im)
    meta = dict(NT0=tuple(int(v) for v in nt_l[0]),
                NT1=tuple(int(v) for v in nt_l[1]),
                has_bias_k=has_bias_k, has_bias_qv=has_bias_qv,
                has_bs=has_bs, has_bc=has_bc, use_bf16=use_bf16)
    return in_maps, meta


# ---------------------------------------------------------------------------
# device program
# ---------------------------------------------------------------------------

def build(meta):
    use_bf16 = meta["use_bf16"]
    parts = meta.get("parts", 4)
    DT = BF16 if use_bf16 else F32
    NT_L = [list(meta["NT0"]), list(meta["NT1"])]
    OFF_L = [np.concatenate([[0], np.cumsum(nt)]).astype(int) for nt in NT_L]
    TOT = [int(o[-1]) for o in OFF_L]
    TMAX = max(max(NT_L[0]), max(NT_L[1]))
    KQC = 4                           # kq psum chunk (tiles, 1 PSUM bank)

    nc = bacc.Bacc("TRN2", target_bir_lowering=False, debug=False, num_devices=R)

    def din(name, shape, dt):
        return nc.dram_tensor(name, shape, dt, kind="ExternalInput").ap()

    t_x_rb = din("x_rb", [R * 128, NPAD], DT)
    t_xT_loc = din("xT_loc", [128, NPAD], DT)
    t_Wk = din("Wk", [NLAYER, H, H], DT)
    t_Wq = din("Wq", [NLAYER, H, H], DT)
    t_Wv = din("Wv", [NLAYER, H, H], DT)
    t_Ws = din("Ws", [NLAYER, H, H], DT)
    t_We_row = din("We_row", [NLAYER, 1, H], DT)
    t_bias_k = din("bias_k", [NLAYER, 128, H], F32)
    t_bias_qv = din("bias_qv", [NLAYER, 128, 2 * H], F32)
    t_bs = din("bs_col", [NLAYER, H, 1], F32)
    t_A = din("A_col", [NLAYER, H, 1], F32)
    t_B = din("B_col", [NLAYER, H, 1], F32)
    t_gidx = [din("gidx0", [128, TOT[0] * 8], I16),
              din("gidx1", [128, TOT[1] * 8], I16)]
    t_S = [din("S0", [128, TOT[0] * 128], DT),
           din("S1", [128, TOT[1] * 128], DT)]
    t_ST = [din("ST0", [128, TOT[0] * 128], DT),
            din("ST1", [128, TOT[1] * 128], DT)]
    t_attr = [din("attr0", [1, TOT[0] * 128], DT),
              din("attr1", [1, TOT[1] * 128], DT)]
    t_IndT = din("IndT", [NPAD, G], DT)
    t_clinT = din("clinT", [NCLIN, G], F32)
    t_Wc_h = din("Wc_h", [H, NCLS], F32)
    t_Wc_c = din("Wc_c", [NCLIN, NCLS], F32)
    t_bc = din("bc_rep", [G, NCLS], F32)

    t_out = nc.dram_tensor("out", [G, NCLS], F32, kind="ExternalOutput").ap()

    qv_lo = nc.dram_tensor("qv_lo", [SPLIT, 2 * H], DT).ap()
    qv_hi = nc.dram_tensor("qv_hi", [NTOT - SPLIT, 2 * H], DT).ap()
    h_loc = [nc.dram_tensor(f"h_loc{l}", [128, NPAD], DT).ap() for l in range(2)]
    ag_out = [nc.dram_tensor(f"ag_out{l}", [R * 128, NPAD], DT,
                             addr_space="Shared").ap() for l in range(2)]
    pool_in = nc.dram_tensor("pool_in", [G, H], F32).ap()
    pool_out = nc.dram_tensor("pool_out", [R * G, H], F32, addr_space="Shared").ap()

    chunks = []
    c0 = 0
    while c0 < NPAD:
        csz = min(512, NPAD - c0)
        chunks.append((c0, csz))
        c0 += csz

    with tile.TileContext(nc) as tc:
        import contextlib
        with contextlib.ExitStack() as ctx:
            consts = ctx.enter_context(tc.tile_pool(name="consts", bufs=1))
            hsb = ctx.enter_context(tc.tile_pool(name="hsb", bufs=1))
            h3p = ctx.enter_context(tc.tile_pool(name="h3p", bufs=1))
            ksb = ctx.enter_context(tc.tile_pool(name="ksb", bufs=1))
            lhp = ctx.enter_context(tc.tile_pool(name="lhp", bufs=4))
            stg = ctx.enter_context(tc.tile_pool(name="stg", bufs=4))
            edg = ctx.enter_context(tc.tile_pool(name="edg", bufs=3))
            sel = ctx.enter_context(tc.tile_pool(name="sel", bufs=3))
            edm = ctx.enter_context(tc.tile_pool(name="edm", bufs=3))
            pnode = ctx.enter_context(tc.tile_pool(name="pnode", bufs=2, space="PSUM"))
            pkq = ctx.enter_context(tc.tile_pool(name="pkq", bufs=2, space="PSUM"))
            pedge = ctx.enter_context(tc.tile_pool(name="pedge", bufs=2, space="PSUM"))
            ppool = ctx.enter_context(tc.tile_pool(name="ppool", bufs=1, space="PSUM"))

            _cid = [0]

            def load_const(src_ap, shape, dt):
                _cid[0] += 1
                t = consts.tile(shape, dt, tag=f"c{_cid[0]}_{src_ap.tensor.name}")
                nc.sync.dma_start(t[:], src_ap)
                return t

            W_t = {}
            for nm, tt in (("k", t_Wk), ("q", t_Wq), ("v", t_Wv), ("s", t_Ws)):
                for l in range(NLAYER):
                    W_t[nm, l] = load_const(tt[l], [H, H], DT)
            We_t = [load_const(t_We_row[l], [1, H], DT) for l in range(NLAYER)]
            bias_k_t = [load_const(t_bias_k[l], [128, H], F32)
                        for l in range(NLAYER)] if meta["has_bias_k"] else None
            bias_qv_t = [load_const(t_bias_qv[l], [128, 2 * H], F32)
                         for l in range(NLAYER)] if meta["has_bias_qv"] else None
            bs_t = [load_const(t_bs[l], [H, 1], F32) for l in range(NLAYER)]
            A_t = [load_const(t_A[l], [H, 1], F32) for l in range(NLAYER)]
            B_t = [load_const(t_B[l], [H, 1], F32) for l in range(NLAYER)]
            ident = consts.tile([128, 128], DT)
            make_identity(nc, ident[:])
            if use_bf16:
                identf = consts.tile([128, 128], F32)
                make_identity(nc, identf[:])
            else:
                identf = ident
            gidx_t = [load_const(t_gidx[0], [128, TOT[0] * 8], I16),
                      load_const(t_gidx[1], [128, TOT[1] * 8], I16)]
            clin_t = load_const(t_clinT, [NCLIN, G], F32)
            Wch_t = load_const(t_Wc_h, [H, NCLS], F32)
            Wcc_t = load_const(t_Wc_c, [NCLIN, NCLS], F32)
            bc_t = load_const(t_bc, [G, NCLS], F32) if meta["has_bc"] else None

            hs = hsb.tile([128, NPAD], F32)        # s + agg accumulator
            k_sb = ksb.tile([128, NGRP, H], DT)    # local k table (node-major)
            h3_prev = None
            h3f = None

            # qv sub-block order: hi rows (>= SPLIT) first, then lo rows
            qv_blocks_hi, qv_blocks_lo = [], []
            for rb in range(R):
                for (c0, csz) in chunks:
                    for s in range(csz // 128):
                        row = rb * NPAD + c0 + s * 128
                        (qv_blocks_hi if row >= SPLIT else qv_blocks_lo).append(
                            (rb, c0 + s * 128))

            def make_runs(blocks):
                # group runs of 4 blocks sharing one lh load (512 cols)
                runs = []
                i = 0
                while i < len(blocks):
                    rb0, r0 = blocks[i]
                    run = [(rb0, r0)]
                    while (len(run) < 4 and i + len(run) < len(blocks)):
                        rbn, rn = blocks[i + len(run)]
                        if rbn == rb0 and rn == run[-1][1] + 128:
                            run.append((rbn, rn))
                        else:
                            break
                    i += len(run)
                    runs.append((rb0, r0, len(run)))
                return runs

            def emit_run(rb0, r0, nrun, l, hsrc):
                    csz = 128 * nrun
                    lh = lhp.tile([128, 512], DT, tag="lh")
                    nc.sync.dma_start(
                        lh[:, 0:csz], hsrc[rb0 * 128:(rb0 + 1) * 128,
                                           r0:r0 + csz])
                    for s in range(nrun):
                        row = rb0 * NPAD + r0 + s * 128
                        ps_full = pnode.tile([128, 512], F32, tag="pn")
                        ps = ps_full[:, 0:2 * H]
                        for jj, nm in enumerate(("q", "v")):
                            nc.tensor.matmul(
                                out=ps[:, jj * H:(jj + 1) * H],
                                lhsT=lh[:, s * 128:(s + 1) * 128],
                                rhs=W_t[nm, l][:], start=True, stop=True)
                        st = stg.tile([128, 2 * H], DT, tag="st")
                        if meta["has_bias_qv"]:
                            nc.vector.tensor_tensor(
                                out=st[:], in0=ps[:], in1=bias_qv_t[l][:],
                                op=OP.add)
                        else:
                            nc.scalar.activation(st[:], ps[:], AF.Copy)
                        if row < SPLIT:
                            nc.sync.dma_start(qv_lo[row:row + 128, :], st[:])
                        else:
                            nc.sync.dma_start(
                                qv_hi[row - SPLIT:row - SPLIT + 128, :], st[:])

            def emit_qv(blocks, l, hsrc):
                for (rb0, r0, nrun) in make_runs(blocks):
                    emit_run(rb0, r0, nrun, l, hsrc)

            def emit_edges(ep, l, filler=None):
                nt_list = NT_L[ep]
                off = OFF_L[ep]
                qv_tab = qv_lo if ep == 0 else qv_hi
                for g in range(NGRP):
                    if filler is not None:
                        filler(g)
                    nt = int(nt_list[g])
                    if nt == 0:
                        continue
                    ne = nt * 128
                    o = int(off[g])
                    gt = edg.tile([128, TMAX, 2 * H], DT, tag="g")
                    nc.gpsimd.dma_gather(
                        gt[:, 0:nt, :], qv_tab[:], gidx_t[ep][:, o * 8:(o + nt) * 8],
                        ne, ne, 2 * H, single_packet=(ne <= 512))
                    St = sel.tile([128, TMAX, 128], DT, tag="S")
                    nc.sync.dma_start(
                        St[:, 0:nt, :],
                        t_S[ep][:, o * 128:(o + nt) * 128].rearrange(
                            "p (t d) -> p t d", t=nt))
                    STt = sel.tile([128, TMAX, 128], DT, tag="ST")
                    nc.sync.dma_start(
                        STt[:, 0:nt, :],
                        t_ST[ep][:, o * 128:(o + nt) * 128].rearrange(
                            "p (t d) -> p t d", t=nt))
                    at = sel.tile([1, TMAX * 128], DT, tag="attr")
                    nc.sync.dma_start(at[:, 0:ne],
                                      t_attr[ep][:, o * 128:o * 128 + ne])
                    ktb = edm.tile([128, TMAX, 128], DT, tag="kt")
                    for q0 in range(0, nt, KQC):
                        qn = min(KQC, nt - q0)
                        pk = pkq.tile([128, KQC, 128], F32, tag="kq")
                        for t in range(q0, q0 + qn):
                            nc.tensor.matmul(
                                out=pk[:, t - q0, :], lhsT=STt[:, t, :],
                                rhs=k_sb[:, g, :], start=True, stop=False)
                            nc.tensor.matmul(
                                out=pk[:, t - q0, :],
                                lhsT=at[0:1, t * 128:(t + 1) * 128],
                                rhs=We_t[l][:], start=False, stop=True)
                        nc.vector.tensor_tensor(
                            out=ktb[:, q0:q0 + qn, :], in0=pk[:, 0:qn, :],
                            in1=gt[:, q0:q0 + qn, 0:H], op=OP.add)
                    nc.scalar.activation(ktb[:, 0:nt, :], ktb[:, 0:nt, :],
                                         AF.Sigmoid)
                    msg = edm.tile([128, TMAX, 128], DT, tag="msg")
                    nc.vector.tensor_tensor(out=msg[:, 0:nt, :],
                                            in0=ktb[:, 0:nt, :],
                                            in1=gt[:, 0:nt, H:2 * H], op=OP.mult)
                    pa = pedge.tile([128, 128], F32, tag="pa")
                    for t in range(nt):
                        nc.tensor.matmul(out=pa[:], lhsT=msg[:, t, :],
                                         rhs=St[:, t, :], start=(t == 0),
                                         stop=(t == nt - 1))
                    nc.vector.tensor_tensor(
                        out=hs[:, g * 128:(g + 1) * 128],
                        in0=hs[:, g * 128:(g + 1) * 128], in1=pa[:], op=OP.add)

            for l in range(NLAYER):
                hsrc = t_x_rb if l == 0 else ag_out[l - 1]

                # ---- s-table (feature-major) into hs + local k table in SBUF
                for (c0, csz) in chunks:
                    if l == 0:
                        rhs_t = lhp.tile([128, 512], DT, tag="lh")
                        nc.sync.dma_start(rhs_t[:, 0:csz], t_xT_loc[:, c0:c0 + csz])
                        rhs_ap = rhs_t[:, 0:csz]
                    else:
                        rhs_ap = h3_prev[:, c0:c0 + csz]
                    pss = pnode.tile([128, 512], F32, tag="pn")
                    nc.tensor.matmul(out=pss[:, 0:csz], lhsT=W_t["s", l][:],
                                     rhs=rhs_ap, start=True, stop=True)
                    if meta["has_bs"]:
                        nc.scalar.activation(hs[:, c0:c0 + csz], pss[:, 0:csz],
                                             AF.Identity, bias=bs_t[l][:], scale=1.0)
                    else:
                        nc.scalar.activation(hs[:, c0:c0 + csz], pss[:, 0:csz],
                                             AF.Copy)
                    for s in range(csz // 128):
                        psk_full = pnode.tile([128, 512], F32, tag="pn")
                        psk = psk_full[:, 0:2 * H]
                        nc.tensor.matmul(out=psk[:, 0:H],
                                         lhsT=rhs_ap[:, s * 128:(s + 1) * 128],
                                         rhs=W_t["k", l][:], start=True, stop=True)
                        gi = c0 // 128 + s
                        if meta["has_bias_k"]:
                            nc.vector.tensor_tensor(out=k_sb[:, gi, :],
                                                    in0=psk[:, 0:H],
                                                    in1=bias_k_t[l][:], op=OP.add)
                        else:
                            nc.scalar.activation(k_sb[:, gi, :], psk[:, 0:H],
                                                 AF.Copy)

                # ---- qv node tables: hi rows first; lo rows interleaved
                # into the ep1 edge phase so PE/DMA build them while Q7 gathers
                emit_qv(qv_blocks_hi, l, hsrc)
                lo_runs = make_runs(qv_blocks_lo)
                pos = [0]
                _l, _hsrc = l, hsrc

                def filler(g, _l=l, _hsrc=hsrc, pos=pos, lo_runs=lo_runs):
                    tgt = (g + 1) * len(lo_runs) // NGRP
                    while pos[0] < min(tgt, len(lo_runs)):
                        rb0, r0, nrun = lo_runs[pos[0]]
                        emit_run(rb0, r0, nrun, _l, _hsrc)
                        pos[0] += 1

                if parts >= 2:
                    emit_edges(1, l, filler)
                while pos[0] < len(lo_runs):
                    rb0, r0, nrun = lo_runs[pos[0]]
                    emit_run(rb0, r0, nrun, l, hsrc)
                    pos[0] += 1
                if parts >= 2:
                    emit_edges(0, l)

                # ---- h update: leaky + BN
                if parts < 3:
                    break
                LCH = 896
                for lc0 in range(0, NPAD, LCH):
                    lsz = min(LCH, NPAD - lc0)
                    sl = slice(lc0, lc0 + lsz)
                    tmp = stg.tile([128, LCH], F32, tag="lrelu")
                    nc.vector.tensor_scalar_mul(tmp[:, 0:lsz], hs[:, sl], SLOPE)
                    nc.vector.tensor_tensor(out=hs[:, sl], in0=hs[:, sl],
                                            in1=tmp[:, 0:lsz], op=OP.max)
                if l < 2:
                    h3 = h3p.tile([128, NPAD], DT)
                    nc.scalar.activation(h3[:], hs[:], AF.Identity,
                                         bias=B_t[l][:], scale=A_t[l][:])
                    nc.sync.dma_start(h_loc[l][:], h3[:])
                    nc.gpsimd.collective_compute(
                        "AllGather", OP.bypass,
                        replica_groups=[list(range(R))],
                        ins=[h_loc[l][:]], outs=[ag_out[l][:]])
                    h3_prev = h3
                else:
                    h3f = hsb.tile([128, NPAD], DT)
                    nc.scalar.activation(h3f[:], hs[:], AF.Identity,
                                         bias=B_t[l][:], scale=A_t[l][:])

            if parts < 4:
                z_dbg = stg.tile([G, NCLS], F32, tag="zsb")
                nc.vector.tensor_copy(z_dbg[:], hs[0:G, 0:NCLS])
                nc.sync.dma_start(t_out[:], z_dbg[:])
            else:
                # ---- pooling
                pp = ppool.tile([G, H], F32)
                for c in range(NGRP):
                    trp = pedge.tile([128, 128], DT, tag="pa")
                    nc.tensor.transpose(out=trp[:], in_=h3f[:, c * 128:(c + 1) * 128],
                                        identity=ident[:])
                    hnode = stg.tile([128, 128], DT, tag="hnode")
                    nc.vector.tensor_copy(hnode[:], trp[:])
                    ind_t = stg.tile([128, G], DT, tag="ind")
                    nc.sync.dma_start(ind_t[:], t_IndT[c * 128:(c + 1) * 128, :])
                    nc.tensor.matmul(out=pp[:], lhsT=ind_t[:], rhs=hnode[:],
                                     start=(c == 0), stop=(c == NGRP - 1))
                pool_sb = stg.tile([G, H], F32, tag="poolsb")
                nc.vector.tensor_copy(pool_sb[:], pp[:])
                nc.sync.dma_start(pool_in[:], pool_sb[:])
                nc.gpsimd.collective_compute(
                    "AllGather", OP.bypass, replica_groups=[list(range(R))],
                    ins=[pool_in[:]], outs=[pool_out[:]])
                pr = stg.tile([G, R, H], F32, tag="pr")
                nc.sync.dma_start(pr[:], pool_out[:].rearrange("(r g) h -> g r h", r=R))
                pooled = stg.tile([G, H], F32, tag="pooled")
                nc.vector.tensor_tensor(out=pooled[:], in0=pr[:, 0, :], in1=pr[:, 1, :],
                                        op=OP.add)
                for r in range(2, R):
                    nc.vector.tensor_tensor(out=pooled[:], in0=pooled[:],
                                            in1=pr[:, r, :], op=OP.add)
                ptp = pedge.tile([H, G], F32, tag="pa")
                nc.tensor.transpose(out=ptp[:], in_=pooled[:], identity=identf[0:G, 0:G])
                pooledT = stg.tile([H, G], F32, tag="pooledT")
                nc.vector.tensor_copy(pooledT[:], ptp[:])
                zp = pedge.tile([G, NCLS], F32, tag="pa")
                nc.tensor.matmul(out=zp[:], lhsT=pooledT[:], rhs=Wch_t[:],
                                 start=True, stop=False)
                nc.tensor.matmul(out=zp[:], lhsT=clin_t[:], rhs=Wcc_t[:],
                                 start=False, stop=True)
                z_sb = stg.tile([G, NCLS], F32, tag="zsb")
                if meta["has_bc"]:
                    nc.vector.tensor_tensor(out=z_sb[:], in0=zp[:], in1=bc_t[:],
                                            op=OP.add)
                else:
                    nc.vector.tensor_copy(z_sb[:], zp[:])
                nc.sync.dma_start(t_out[:], z_sb[:])

    nc.compile()
    return nc


# ---------------------------------------------------------------------------

_CACHE = {}


def kernel(**inputs):
    in_maps, meta = prep(inputs)
    key = tuple(sorted((k, v) for k, v in meta.items()))
    if key not in _CACHE:
        _CACHE[key] = build(meta)
    nc = _CACHE[key]
    res = run_bass_kernel_spmd(nc, in_maps, list(range(R)))
    return np.asarray(res.results[0]["out"], np.float32)


def kernel_profiled(**inputs):
    """Like kernel() but also returns (exec_time_ns, trace_path)."""
    in_maps, meta = prep(inputs)
    key = tuple(sorted((k, v) for k, v in meta.items()))
    if key not in _CACHE:
        _CACHE[key] = build(meta)
    nc = _CACHE[key]
    res = run_bass_kernel_spmd(nc, in_maps, list(range(R)), trace=True)
    out = np.asarray(res.results[0]["out"], np.float32)
    trace_path = None
    if res.instructions_and_trace is not None:
        trace_path = res.instructions_and_trace[1]
    return out, res.exec_time_ns, trace_path


if __name__ == "__main__":
    pass
